# revision 1
# baseline (speedup 1.0000x reference)
"""Trainium2 Bass kernel for the OFPenalty eigenvalue-penalty loss.

Math (per sample b of 256):
  W = x[b] reshaped [C=2048, N=49];  G = W^T W  (49x49 Gram matrix)
  run1: x9 = G^9 x0 (power iteration, normalization deferred - scale
        invariant), largest = Rayleigh(G, x9) = x9^T G x9 / x9^T x9
  run2: B = G - largest*I, u9 = B^9 x1 (x1 = scaled x9),
        tmp = Rayleigh(B, u9); smallest = tmp + largest
  penalty = (largest/smallest - 1)^2 ; output = mean over batch.

Sharding: pure data parallel, 32 samples per core on 8 cores.  Samples
are processed in pairs packed block-diagonally: sample 2p lives on
partitions 0:49, sample 2p+1 on partitions 64:113 (the gap keeps every
compute-engine access 32-partition aligned).  Scalings by powers of two
(exact) keep the unnormalized power iterates inside fp32 range.
Rayleigh numerators/denominators are columnwise dot products: masked
elementwise multiply on VectorE, then a ones-vector matmul reduces over
partitions, leaving per-sample scalars in free-dim rows.
"""

import os
import sys
from contextlib import ExitStack

import numpy as np

for _p in ("/opt/trn_rl_repo",):
    if os.path.isdir(_p) and _p not in sys.path:
        sys.path.insert(0, _p)

import concourse.bass as bass  # noqa: E402
import concourse.tile as tile  # noqa: E402
from concourse import bacc, mybir  # noqa: E402
from concourse.bass_utils import run_bass_kernel_spmd  # noqa: E402

F32 = mybir.dt.float32
I32 = mybir.dt.int32
ALU = mybir.AluOpType

B, C, N = 256, 2048, 49
NCORES = 8
BS = B // NCORES  # 32 samples per core
NPAIR = BS // 2  # 16 pairs
KT = C // 128  # 16 contraction tiles
PG = 128  # gapped pair-vector space: blocks at [0:49], [64:113]
B1 = 64  # partition base of the second sample in a pair
S52 = float(2.0**-52)  # rescale before Rayleigh products
S102 = float(2.0**-102)  # rescale x9 -> x1 (run2 warm start)
NITER = 9


def _chain_waves(nc, stats, v0, nsteps, vpool, pspool, label, lamv=None):
    """Apply per-pair matrices nsteps times to all NPAIR columns in lockstep.

    Each wave issues one matvec per pair into columns of a shared PSUM
    tile; one batched fixup/copy feeds the next wave.  With lamv given,
    the matrices act as shifted A - lam*I without materializing them:
    the inter-wave step computes nxt = psum - lamv*cur (the lamv*cur
    product is issued before the matvecs so it hides under them).

    Returns (last_sbuf, last_psum, last_t): the input vector of the
    final wave (SBUF), the final wave's raw A*v PSUM, and the final
    wave's lamv*cur product (None when lamv is None).
    """
    cur = v0
    psw = None
    last_sbuf = None
    t = None
    for i in range(nsteps):
        psw = pspool.tile([PG, NPAIR], F32, tag="mvw", name=f"mvw_{label}{i}")
        if lamv is not None:
            t = vpool.tile([PG, NPAIR], F32, tag="vt", name=f"vt_{label}{i}")
            nc.vector.tensor_mul(t[:], lamv, cur)
        for p in range(NPAIR):
            nc.tensor.matmul(
                psw[:, p : p + 1], stats[p], cur[:, p : p + 1],
                start=True, stop=True,
            )
        if i < nsteps - 1:
            nxt = vpool.tile([PG, NPAIR], F32, tag="vw", name=f"vw_{label}{i}")
            if lamv is not None:
                nc.vector.tensor_sub(nxt[:], psw[:], t[:])
            else:
                nc.vector.tensor_copy(nxt[:], psw[:])
            last_sbuf = nxt
            cur = nxt[:]
    return last_sbuf, psw, t


def _emit(tc, x, x0, pen, repeat=1):
    nc = tc.nc
    ctx = ExitStack()
    with ctx:
        const = ctx.enter_context(tc.tile_pool(name="const", bufs=1))
        xpool = ctx.enter_context(tc.tile_pool(name="xt", bufs=4))
        vpool = ctx.enter_context(tc.tile_pool(name="vec", bufs=3))
        ps_ata = ctx.enter_context(tc.tile_pool(name="ps_ata", bufs=4, space="PSUM"))
        ps_mv = ctx.enter_context(tc.tile_pool(name="ps_mv", bufs=3, space="PSUM"))
        ps_msc = ctx.enter_context(tc.tile_pool(name="ps_msc", bufs=1, space="PSUM"))

        # ---- constants -------------------------------------------------
        # x0 columns: X0[0:49, p] = x0[2p], X0[64:113, p] = x0[2p+1]
        X0 = const.tile([PG, NPAIR], F32)
        nc.gpsimd.memset(X0[:], 0.0)
        x0r = x0.rearrange("(p two) j -> two j p", two=2)
        nc.sync.dma_start(X0[0:N, :], x0r[0])
        nc.sync.dma_start(X0[B1 : B1 + N, :], x0r[1])

        # identity mask (used to build B = A - lambda*I)
        DIAG = const.tile([PG, PG], F32)
        nc.gpsimd.memset(DIAG[:], 0.0)
        nc.gpsimd.affine_select(
            out=DIAG[:],
            in_=DIAG[:],
            compare_op=ALU.not_equal,
            fill=1.0,
            base=0,
            pattern=[[-1, PG]],
            channel_multiplier=1,
        )

        # block-ownership row masks: CM0 = 1 on partitions of sample 0's
        # block (cols 0:49), CM1 on sample 1's block (cols 64:113)
        CM0 = const.tile([1, PG], F32)
        nc.gpsimd.memset(CM0[:], 0.0)
        nc.gpsimd.memset(CM0[:, 0:N], 1.0)
        CM1 = const.tile([1, PG], F32)
        nc.gpsimd.memset(CM1[:], 0.0)
        nc.gpsimd.memset(CM1[:, B1 : B1 + N], 1.0)

        ONE128 = const.tile([PG, 1], F32)
        nc.gpsimd.memset(ONE128[:], 1.0)

        # ---- persistent intermediates ---------------------------------
        X9M = const.tile([PG, BS], F32)  # block-masked x9, col per sample
        WF1 = const.tile([PG, NPAIR], F32)  # w columns, one per pair
        XF1 = const.tile([PG, NPAIR], F32)  # scaled x9 columns, one per pair
        U9M = const.tile([PG, BS], F32)
        WF2 = const.tile([PG, NPAIR], F32)
        XF2 = const.tile([PG, NPAIR], F32)
        X1A = const.tile([PG, NPAIR], F32)  # run2 warm starts
        LAMV = const.tile([PG, NPAIR], F32)  # lambda per partition
        nc.gpsimd.memset(X9M[:], 0.0)
        nc.gpsimd.memset(U9M[:], 0.0)
        Aall = const.tile([PG, NPAIR, PG], F32)  # blockdiag Gram per pair
        Ball = const.tile([PG, NPAIR, PG], F32)  # shifted matrices
        lamI = const.tile([PG, NPAIR, PG], F32)

        for _rep in range(repeat):
            # ---- phase 1: Gram matrices -----------------------------------
            # Partition q holds c-rows {512b + 4q + r : r<4}, sample-major in
            # SBUF: 784B-contiguous DMA descriptors (>=512B keeps DMA at full
            # bandwidth) AND contiguous [128, 49] matmul stationaries.
            # Sample 1's Gram accumulates into psum partitions 64:113 (PE
            # column-group 64), so downstream block layout is unchanged.
            xrs = x.rearrange(
                "(p two) (b q r) j -> p two q b (r j)", two=2, b=4, q=128, r=4
            )
            nc.gpsimd.memset(Aall[:], 0.0)
            As = []
            for p in range(NPAIR):
                xt = xpool.tile([128, 2, KT * N], F32, tag="xt", name=f"xt{p}")
                for s in range(2):
                    eng = nc.sync if s == 0 else nc.scalar
                    eng.dma_start(
                        xt[:, s, :].rearrange("q (b m) -> q b m", b=4),
                        xrs[p, s],
                    )
                # interleave the two samples' accumulation groups: they
                # occupy PE column-groups 0 and 64 (and separate PSUM
                # banks), so adjacent matmuls can overlap in the array
                psa = ps_ata.tile([PG, N], F32, tag="ata", name=f"ata{p}a")
                psb = ps_ata.tile([PG, N], F32, tag="ata", name=f"ata{p}b")
                for k in range(KT):
                    for s in range(2):
                        pst = psa if s == 0 else psb
                        ob = 0 if s == 0 else B1
                        wk = xt[:, s, k * N : (k + 1) * N]
                        nc.tensor.matmul(
                            pst[ob : ob + N, :],
                            wk,
                            wk,
                            start=(k == 0),
                            stop=(k == KT - 1),
                        )
                A = Aall[:, p, :]
                nc.scalar.copy(A[0:N, 0:N], psa[0:N, :])
                nc.scalar.copy(A[B1 : B1 + N, B1 : B1 + N], psb[B1 : B1 + N, :])
                As.append(A)

            # ---- phase 2: run1 chains (wave-major across pairs) -----------
            # 10 waves: wave 9's input x9 (unscaled SBUF) and wave 10's
            # output w = A*x9 (raw PSUM) come out of the same chain.
            x9u, psw1, _ = _chain_waves(nc, As, X0[:], NITER + 1, vpool, ps_mv, "a")
            x9e = X9M.rearrange("q (p j) -> q p j", j=2)
            nc.vector.tensor_scalar(WF1[:], psw1[:], S52, None, op0=ALU.mult)
            nc.vector.tensor_scalar(XF1[:], x9u[:], S52, None, op0=ALU.mult)
            nc.vector.tensor_scalar(
                x9e[0:N, :, 0], x9u[0:N, :], S52, None, op0=ALU.mult
            )
            nc.vector.tensor_scalar(
                x9e[B1 : B1 + N, :, 1], x9u[B1 : B1 + N, :], S52, None, op0=ALU.mult
            )
            nc.vector.tensor_scalar(X1A[:], x9u[:], S102, None, op0=ALU.mult)

            def rayleigh_rows(X9M_, WF_, XF_, ndrow, label):
                # T[:, 0:32] = X9M * w(dup per sample); T[:, 32:64] = X9M * x9(dup)
                T = const.tile([PG, 2 * BS], F32, name=f"T{label}")
                wdup = WF_[:, :, None].broadcast_to([PG, NPAIR, 2])
                xdup = XF_[:, :, None].broadcast_to([PG, NPAIR, 2])
                nc.vector.tensor_mul(T[:, 0:BS], X9M_[:], wdup)
                nc.vector.tensor_mul(T[:, BS : 2 * BS], X9M_[:], xdup)
                pnd = ps_msc.tile([1, 2 * BS], F32, tag="msc", name=f"pnd{label}")
                nc.tensor.matmul(pnd[:], ONE128[:], T[:], start=True, stop=True)
                nc.scalar.copy(ndrow[:], pnd[:])

            # ---- Rayleigh 1 (num/den rows) + lambda broadcast -------------
            ND1 = const.tile([1, 2 * BS], F32)  # [num row | den row]
            rayleigh_rows(X9M, WF1, XF1, ND1, "r1")
            LAMR = const.tile([1, BS], F32)
            RDR = const.tile([1, BS], F32)
            nc.vector.reciprocal(RDR[:], ND1[:, BS : 2 * BS])
            nc.vector.tensor_mul(LAMR[:], ND1[:, 0:BS], RDR[:])

            # LAMV[q, p] = lambda of the sample owning partition q in pair p,
            # as two accumulating rank-1 matmuls: CM0^T lam_even + CM1^T lam_odd
            lam2 = LAMR.rearrange("o (p h) -> o p h", h=2)
            psl = ps_msc.tile([PG, NPAIR], F32, tag="msc", name="psl")
            nc.tensor.matmul(psl[:], CM0[:], lam2[:, :, 0], start=True, stop=False)
            nc.tensor.matmul(psl[:], CM1[:], lam2[:, :, 1], start=False, stop=True)
            nc.scalar.copy(LAMV[:], psl[:])

            # ---- phase 3: shifted matrices + run2 chains ------------------
            # Ball = Aall - lam*I in two batched VectorE ops (materialized:
            # computing A*v - lam*v per wave instead amplifies the PE's
            # 2-pass fp32 rounding through cancellation; hw err 1.8e-4 vs
            # 2.5e-5 materialized).  Wave 10 gives w2 = B*u9 directly.
            H8 = NPAIR // 2
            db = DIAG[:, None, :].broadcast_to([PG, NPAIR, PG])
            lb = LAMV[:, :, None].broadcast_to([PG, NPAIR, PG])
            nc.vector.tensor_tensor(
                lamI[:, 0:H8], db[:, 0:H8], lb[:, 0:H8], op=ALU.mult
            )
            nc.gpsimd.tensor_tensor(
                lamI[:, H8:NPAIR], db[:, H8:NPAIR], lb[:, H8:NPAIR], op=ALU.mult
            )
            nc.vector.tensor_sub(Ball[:, 0:H8], Aall[:, 0:H8], lamI[:, 0:H8])
            nc.vector.tensor_sub(
                Ball[:, H8:NPAIR], Aall[:, H8:NPAIR], lamI[:, H8:NPAIR]
            )
            Bs = [Ball[:, p, :] for p in range(NPAIR)]
            u9u, psw2, _ = _chain_waves(
                nc, Bs, X1A[:], NITER + 1, vpool, ps_mv, "b"
            )
            u9e = U9M.rearrange("q (p j) -> q p j", j=2)
            nc.vector.tensor_scalar(WF2[:], psw2[:], S52, None, op0=ALU.mult)
            nc.vector.tensor_scalar(XF2[:], u9u[:], S52, None, op0=ALU.mult)
            nc.vector.tensor_scalar(
                u9e[0:N, :, 0], u9u[0:N, :], S52, None, op0=ALU.mult
            )
            nc.vector.tensor_scalar(
                u9e[B1 : B1 + N, :, 1], u9u[B1 : B1 + N, :], S52, None, op0=ALU.mult
            )

            # ---- Rayleigh 2 + penalty (all on free-dim rows) --------------
            ND2 = const.tile([1, 2 * BS], F32)
            rayleigh_rows(U9M, WF2, XF2, ND2, "r2")
            RD2 = const.tile([1, BS], F32)
            TMP = const.tile([1, BS], F32)
            SM = const.tile([1, BS], F32)
            RS = const.tile([1, BS], F32)
            RT = const.tile([1, BS], F32)
            PEN = const.tile([1, BS], F32)
            # (largest/smallest - 1)^2 == (tmp/smallest)^2  (largest-smallest=-tmp)
            nc.vector.reciprocal(RD2[:], ND2[:, BS : 2 * BS])
            nc.vector.tensor_mul(TMP[:], ND2[:, 0:BS], RD2[:])
            nc.vector.tensor_add(SM[:], TMP[:], LAMR[:])
            nc.vector.reciprocal(RS[:], SM[:])
            nc.vector.tensor_mul(RT[:], TMP[:], RS[:])
            nc.vector.tensor_mul(PEN[:], RT[:], RT[:])
            nc.sync.dma_start(pen, PEN[:])


_NC_CACHE = {}


def build_nc(repeat=1):
    if repeat in _NC_CACHE:
        return _NC_CACHE[repeat]
    nc = bacc.Bacc("TRN2", target_bir_lowering=False, debug=False)
    x = nc.dram_tensor("x", [BS, C, N], F32, kind="ExternalInput")
    x0 = nc.dram_tensor("x0", [BS, N], F32, kind="ExternalInput")
    pen = nc.dram_tensor("pen", [BS], F32, kind="ExternalOutput")
    with tile.TileContext(nc) as tc:
        _emit(tc, x.ap(), x0.ap(), pen.ap(), repeat=repeat)
    nc.compile()
    _NC_CACHE[repeat] = nc
    return nc


LAST_RESULTS = None


def kernel(x, x0):
    global LAST_RESULTS
    x = np.ascontiguousarray(np.asarray(x, dtype=np.float32).reshape(B, C, N))
    x0 = np.ascontiguousarray(np.asarray(x0, dtype=np.float32).reshape(B, N))
    nc = build_nc()
    in_maps = [
        {"x": x[i * BS : (i + 1) * BS], "x0": x0[i * BS : (i + 1) * BS]}
        for i in range(NCORES)
    ]
    trace = bool(int(os.environ.get("KERNEL_TRACE", "0")))
    res = run_bass_kernel_spmd(nc, in_maps, list(range(NCORES)), trace=trace)
    LAST_RESULTS = res
    pens = np.concatenate([r["pen"].reshape(-1) for r in res.results])
    return np.float32(pens.sum(dtype=np.float64) / B)



# revision 36
# speedup vs baseline: 1.9107x; 1.9107x over previous
"""Trainium2 Bass kernel for the OFPenalty eigenvalue-penalty loss.

Math (per sample b of 256):
  W = x[b] reshaped [C=2048, N=49];  G = W^T W  (49x49 Gram matrix)
  run1: x9 = G^9 x0 (power iteration, normalization deferred - scale
        invariant), largest = Rayleigh(G, x9) = x9^T (G x9) / x9^T x9
  run2: B = G - largest*I, u9 = B^9 x9, tmp = Rayleigh(B, u9)
  penalty = (tmp / (tmp + largest))^2 ; output = mean over batch.

Distribution: pure data parallel, 32 samples per core on 8 cores.
Samples are processed in pairs packed block-diagonally: sample 2p on
partitions 0:49, sample 2p+1 on partitions 64:113.

Performance structure (per core):
  - x is staged to fp16 on the host; the device streams 6.4MB instead of
    12.8MB, halving the DMA floor to ~18us.  One DMA per pair (3136B per
    partition) keeps descriptors at full bus efficiency.
  - Gram matmuls run in fp16 (1 PE cycle/row instead of fp32's 4); the
    gram phase + per-pair A^2 squaring builds (fp32, deferred two pairs
    so their operands are always ready) stream at DMA pace with the
    in-order PE queue free of dependency stalls.  ~3.5us of dummy
    matmuls at t=0 ramp the PE to peak clock before the first gram.
  - The power-iteration chains stay in fp32 (the per-sample penalty is
    ill-conditioned; low-precision trajectories diverge) and run as ONE
    16-pair lockstep chain of 15 dependency levels after the grams:
      run1:  x9 = A2 A2 A2 A2 (A x0)  (5 wave levels, 2^-12 per copy)
      run2:  u9 via 4 distributed B^2 double-steps, all three terms
             (A2 u, A(-2 lam 2^-12 u), I(lam^2 2^-12 u)) accumulated in
             PSUM so each level needs only one PSUM->SBUF copy;
      Rayleigh quotients use per-sample block dots via a [128,2] mask
      matmul; w2 = A u9 with lam folded into the final divide; the
      divide + penalty run fused on the vector engine.
    Exact powers of two cancel in every Rayleigh quotient.
"""

import os
import sys
from contextlib import ExitStack

import numpy as np

for _p in ("/opt/trn_rl_repo",):
    if os.path.isdir(_p) and _p not in sys.path:
        sys.path.insert(0, _p)

import concourse.bass as bass  # noqa: E402  (import keeps bass registered)
import concourse.tile as tile  # noqa: E402
from concourse import bacc, mybir  # noqa: E402
from concourse.bass_utils import run_bass_kernel_spmd  # noqa: E402

F32 = mybir.dt.float32
F16 = mybir.dt.float16
ALU = mybir.AluOpType

B, C, N = 256, 2048, 49
NCORES = 8
BS = B // NCORES  # 32 samples per core
NPAIR = BS // 2  # 16 pairs
KT = C // 128  # 16 contraction tiles
PG = 128
B1 = 64  # partition base of the second sample in a pair
S12 = float(2.0**-12)  # per-copy rescale (exact; cancels in Rayleigh)
S30 = float(2.0**-30)  # Rayleigh-2 product rescale (cancels exactly)
W = NPAIR  # chain width: all pairs in lockstep


def _emit(tc, x16, x0c, m2t, pen):
    nc = tc.nc
    ctx = ExitStack()
    with ctx:
        const = ctx.enter_context(tc.tile_pool(name="const", bufs=1))
        xpool = ctx.enter_context(tc.tile_pool(name="xt", bufs=5))
        vpool = ctx.enter_context(tc.tile_pool(name="vec", bufs=3))
        ps_ata = ctx.enter_context(tc.tile_pool(name="ps_ata", bufs=4, space="PSUM"))
        ps_bld = ctx.enter_context(tc.tile_pool(name="ps_bld", bufs=2, space="PSUM"))
        ps_wv = ctx.enter_context(tc.tile_pool(name="ps_wv", bufs=2, space="PSUM"))

        # ---- constants -------------------------------------------------
        X0 = const.tile([PG, NPAIR], F32)

        # PE p-state warmup: ~3.5us of dummy matmuls before the first gram
        # arrives, so the tensor engine is at peak clock from pair 0 on.
        JNK = const.tile([PG, PG], F16)
        nc.gpsimd.memset(JNK[:], 0.0)
        for i in range(34):
            psj = ps_bld.tile([PG, PG], F32, tag="bld", name=f"warm{i}")
            nc.tensor.matmul(psj[:], JNK[:], JNK[:], start=True, stop=True)

        IDN = const.tile([PG, PG], F32)
        nc.gpsimd.memset(IDN[:], 0.0)
        nc.gpsimd.affine_select(
            out=IDN[:],
            in_=IDN[:],
            compare_op=ALU.not_equal,
            fill=1.0,
            base=0,
            pattern=[[-1, PG]],
            channel_multiplier=1,
        )

        CM01 = const.tile([PG, 2], F32)
        nc.gpsimd.memset(CM01[:], 0.0)
        nc.gpsimd.memset(CM01[0:N, 0:1], 1.0)
        nc.gpsimd.memset(CM01[B1 : B1 + N, 1:2], 1.0)

        M2T = const.tile([2, PG], F32)

        At = [const.tile([PG, PG], F32, tag=f"A{p}", name=f"A{p}")
              for p in range(NPAIR)]
        for p in range(NPAIR):
            nc.gpsimd.memset(At[p][:], 0.0)
        A2t = [const.tile([PG, PG], F32, tag=f"A2_{p}", name=f"A2_{p}")
               for p in range(NPAIR)]
        PENT = const.tile([2, NPAIR], F32)

        # ---- gram phase + pipelined A2 builds --------------------------
        # Two pairs per DMA (6272B per partition).  Per pair: fp16 gram
        # matmuls into two PSUM accumulators (PE column groups 0 and 64),
        # fp32 copies into the block-diagonal A tile, then A2 = (A@A)*2^-12
        # two pairs later so every matmul's operands are long ready before
        # the in-order PE reaches it.
        def emit_sq(p):
            ps2 = ps_bld.tile([PG, PG], F32, tag="bld", name=f"a2m{p}")
            nc.tensor.matmul(ps2[:], At[p][:], At[p][:], start=True, stop=True)
            if p % 2 == 0:
                nc.vector.tensor_scalar(A2t[p][:], ps2[:], S12, None,
                                        op0=ALU.mult)
            else:
                nc.scalar.mul(A2t[p][:], ps2[:], S12)

        for d in range(NPAIR // 2):
            xt = xpool.tile([PG, 4 * KT * N], F16, tag="xt", name=f"xt{d}")
            nc.sync.dma_start(xt[:], x16[d])
            if d == 0:
                nc.scalar.dma_start(X0[:], x0c)
                nc.scalar.dma_start(M2T[:], m2t)
            for h in range(2):
                p = 2 * d + h
                psa = ps_ata.tile([PG, N], F32, tag="ata", name=f"ata{p}a")
                psb = ps_ata.tile([PG, N], F32, tag="ata", name=f"ata{p}b")
                for k in range(KT):
                    for s in range(2):
                        pst = psa if s == 0 else psb
                        ob = 0 if s == 0 else B1
                        off = (2 * h + s) * (KT * N)
                        wk = xt[:, off + k * N : off + (k + 1) * N]
                        nc.tensor.matmul(
                            pst[ob : ob + N, :], wk, wk,
                            start=(k == 0), stop=(k == KT - 1),
                        )
                A = At[p][:]
                nc.vector.tensor_copy(A[0:N, 0:N], psa[0:N, :])
                nc.scalar.copy(A[B1 : B1 + N, B1 : B1 + N], psb[B1 : B1 + N, :])
                if p >= 2:
                    emit_sq(p - 2)
        for p in range(NPAIR - 2, NPAIR):
            emit_sq(p)

        Av = [At[p][:] for p in range(NPAIR)]
        A2v = [A2t[p][:] for p in range(NPAIR)]

        # ---- chain helpers ---------------------------------------------
        def chain_ps(name):
            return ps_wv.tile([PG, 2 * W], F32, tag="wv", name=name)

        def vtile(tag, name=None):
            return vpool.tile([PG, W], F32, tag=tag, name=name or tag)

        def matvecs(views, cur, tag):
            psw = chain_ps(f"wv_{tag}")
            for j in range(W):
                nc.tensor.matmul(psw[:, j : j + 1], views[j], cur[:, j : j + 1],
                                 start=True, stop=True)
            return psw

        st = {}

        # ---- run 1: x9 = A4 A4 (A x0), *2^-12 per wave -----------------
        def wave(views, src, dst, eng, tag):
            psw = matvecs(views, st[src], tag)
            nxt = vtile("v", f"v_{dst}")
            if eng == 0:
                nc.vector.tensor_scalar(nxt[:], psw[:, 0:W], S12, None,
                                        op0=ALU.mult)
            else:
                nc.scalar.mul(nxt[:], psw[:, 0:W], S12)
            st[dst] = nxt[:]

        st["x0"] = X0[:]
        wave(Av, "x0", "v1", 0, "v1")
        wave(A2v, "v1", "v3", 1, "v3")
        wave(A2v, "v3", "v5", 0, "v5")
        wave(A2v, "v5", "v7", 1, "v7")
        wave(A2v, "v7", "x9", 0, "x9")

        # ---- Rayleigh 1 -> LAM -----------------------------------------
        TT1 = vpool.tile([PG, 2 * W], F32, tag="tt", name="tt_r1")
        nc.scalar.square(TT1[:, W : 2 * W], st["x9"])
        psww = matvecs(Av, st["x9"], "w")
        nc.vector.tensor_mul(TT1[:, 0:W], st["x9"], psww[:, 0:W])

        pd1 = chain_ps("pd_r1")
        nc.tensor.matmul(pd1[0:2, :], CM01[:], TT1[:], start=True, stop=True)

        ND1 = vpool.tile([2, 2 * W], F32, tag="nd", name="nd_r1")
        RD1 = vpool.tile([2, W], F32, tag="rd", name="rd_r1")
        LAM = vpool.tile([2, W], F32, tag="lam", name="lam")
        nc.vector.tensor_copy(ND1[:], pd1[0:2, :])
        nc.vector.reciprocal(RD1[:], ND1[:, W : 2 * W])
        nc.vector.tensor_mul(LAM[:], ND1[:, 0:W], RD1[:])

        # ---- lambda broadcast to partitions ----------------------------
        psl = chain_ps("psl")
        nc.tensor.matmul(psl[:, 0:W], M2T[:], LAM[:], start=True, stop=True)

        # ---- run 2: u1 = A x9 - lam x9, then 4 distributed B^2 steps ---
        # u_{k+2} = (A2 u) + A(-2 lam S12 u) + I(lam^2 S12 u)
        #         = B^2 u * 2^-12   (A2 carries one 2^-12 already);
        # each level: 2 DVE pre-products, 3 accumulated matvecs per pair,
        # one PSUM->SBUF copy.
        LVN = vtile("lv", "lvn")
        nc.vector.tensor_scalar(LVN[:], psl[:, 0:W], -1.0, None, op0=ALU.mult)
        hx = vtile("t1", "hx")
        nc.vector.tensor_mul(hx[:], LVN[:], st["x9"])
        psu1 = chain_ps("wv_u1")
        for j in range(W):
            nc.tensor.matmul(psu1[:, j : j + 1], Av[j],
                             st["x9"][:, j : j + 1], start=True, stop=False)
            nc.tensor.matmul(psu1[:, j : j + 1], IDN[:],
                             hx[:, j : j + 1], start=False, stop=True)
        u1 = vtile("v", "u1")
        nc.vector.tensor_copy(u1[:], psu1[:, 0:W])
        st["u1"] = u1[:]
        NLV2S = vtile("lv2", "nlv2s")
        nc.vector.tensor_scalar(NLV2S[:], LVN[:], 2.0 * S12, None, op0=ALU.mult)
        SQl = vtile("sq", "sql")
        nc.vector.tensor_mul(SQl[:], LVN[:], LVN[:])
        LVQS = vtile("lvq", "lvqs")
        nc.vector.tensor_scalar(LVQS[:], SQl[:], S12, None, op0=ALU.mult)

        for i, (src, dst) in enumerate(
            [("u1", "u3"), ("u3", "u5"), ("u5", "u7"), ("u7", "u9")]
        ):
            gv = vtile("t1", f"g_{dst}")
            nc.vector.tensor_mul(gv[:], LVQS[:], st[src])
            hv = vtile("e", f"h_{dst}")
            nc.vector.tensor_mul(hv[:], NLV2S[:], st[src])
            psw = chain_ps(f"wv_{dst}")
            for j in range(W):
                nc.tensor.matmul(psw[:, j : j + 1], A2v[j],
                                 st[src][:, j : j + 1], start=True, stop=False)
                nc.tensor.matmul(psw[:, j : j + 1], Av[j],
                                 hv[:, j : j + 1], start=False, stop=False)
                nc.tensor.matmul(psw[:, j : j + 1], IDN[:],
                                 gv[:, j : j + 1], start=False, stop=True)
            nxt = vtile("v", dst)
            if i % 2 == 0:
                nc.vector.tensor_copy(nxt[:], psw[:, 0:W])
            else:
                nc.scalar.copy(nxt[:], psw[:, 0:W])
            st[dst] = nxt[:]

        # ---- Rayleigh 2 (w2 = A u9; lam folded into the divide) --------
        US = vtile("us", "us")
        nc.scalar.mul(US[:], st["u9"], S30)
        psw2 = matvecs(Av, st["u9"], "w2")
        TT2 = vpool.tile([PG, 2 * W], F32, tag="tt", name="tt_r2")
        WS = vtile("ws", "ws")
        nc.vector.tensor_scalar(WS[:], psw2[:, 0:W], S30, None, op0=ALU.mult)
        nc.scalar.square(TT2[:, W : 2 * W], US[:])
        nc.vector.tensor_mul(TT2[:, 0:W], US[:], WS[:])

        pd2 = chain_ps("pd_r2")
        nc.tensor.matmul(pd2[0:2, :], CM01[:], TT2[:], start=True, stop=True)

        # tmp = u9.A.u9/u9.u9 - lam ; sm = tmp + lam; pen = (tmp/sm)^2
        ND2 = vpool.tile([2, 2 * W], F32, tag="nd", name="nd_r2")
        RD2 = vpool.tile([2, W], F32, tag="rd", name="rd_r2")
        T0 = vpool.tile([2, W], F32, tag="t0", name="t0")
        DF = vpool.tile([2, W], F32, tag="df", name="df")
        RS = vpool.tile([2, W], F32, tag="rs", name="rs")
        RT = vpool.tile([2, W], F32, tag="rt", name="rt")
        nc.vector.tensor_copy(ND2[:], pd2[0:2, :])
        nc.vector.reciprocal(RD2[:], ND2[:, W : 2 * W])
        nc.vector.tensor_mul(T0[:], ND2[:, 0:W], RD2[:])
        nc.vector.tensor_sub(DF[:], T0[:], LAM[:])
        nc.vector.reciprocal(RS[:], T0[:])
        nc.vector.tensor_mul(RT[:], DF[:], RS[:])
        nc.vector.tensor_mul(PENT[:], RT[:], RT[:])

        nc.sync.dma_start(pen, PENT[:])


_NC_CACHE = {}


def build_nc():
    if "nc" in _NC_CACHE:
        return _NC_CACHE["nc"]
    nc = bacc.Bacc("TRN2", target_bir_lowering=False, debug=False)
    x16 = nc.dram_tensor("x16", [NPAIR // 2, PG, 4 * KT * N], F16, kind="ExternalInput")
    x0c = nc.dram_tensor("x0c", [PG, NPAIR], F32, kind="ExternalInput")
    m2t = nc.dram_tensor("m2t", [2, PG], F32, kind="ExternalInput")
    pen = nc.dram_tensor("pen", [2, NPAIR], F32, kind="ExternalOutput")
    with tile.TileContext(nc) as tc:
        _emit(tc, x16.ap(), x0c.ap(), m2t.ap(), pen.ap())
    nc.compile()
    _NC_CACHE["nc"] = nc
    return nc


LAST_RESULTS = None


def kernel(x, x0):
    global LAST_RESULTS
    x = np.asarray(x, dtype=np.float32).reshape(B, C, N)
    x0 = np.asarray(x0, dtype=np.float32).reshape(B, N)

    # host staging: fp16 pair tiles in gram k-tile layout.
    # channel c = 512*b + 4*q + r -> partition q holds, per sample, the
    # 784 values [b, r, j] contiguously (3136B descriptors).
    xr = x.reshape(NCORES, NPAIR // 2, 4, 4, PG, 4, N)
    x16 = np.ascontiguousarray(
        xr.transpose(0, 1, 4, 2, 3, 5, 6).reshape(
            NCORES, NPAIR // 2, PG, 4 * KT * N
        )
    ).astype(np.float16)

    x0r = x0.reshape(NCORES, NPAIR, 2, N)
    x0c = np.zeros((NCORES, PG, NPAIR), dtype=np.float32)
    x0c[:, 0:N, :] = x0r[:, :, 0, :].transpose(0, 2, 1)
    x0c[:, B1 : B1 + N, :] = x0r[:, :, 1, :].transpose(0, 2, 1)

    m2t = np.zeros((2, PG), dtype=np.float32)
    m2t[0, 0:N] = 1.0
    m2t[1, B1 : B1 + N] = 1.0
    nc = build_nc()
    in_maps = [
        {"x16": x16[i], "x0c": x0c[i], "m2t": m2t} for i in range(NCORES)
    ]
    trace = bool(int(os.environ.get("KERNEL_TRACE", "0")))
    res = run_bass_kernel_spmd(nc, in_maps, list(range(NCORES)), trace=trace)
    LAST_RESULTS = res
    # pen[s, p] = penalty of sample 2p+s on that core
    pens = np.concatenate(
        [r["pen"].reshape(2, NPAIR).T.reshape(-1) for r in res.results]
    )
    return np.float32(pens.sum(dtype=np.float64) / B)
